# revision 1
# baseline (speedup 1.0000x reference)
"""Trainium2 Bass kernel for nn_Attention_87625922773171.

Spatial-reduction attention (PVT-style) over B=4, N=5120 (1024 template +
4096 search tokens), C=256, 8 heads, sr_ratio=2.

Sharding: 8 cores = 4 batches x 2 head-groups (4 heads each).  Each core
computes its (b, hg) slice end-to-end in a transposed ("channels on
partitions") layout; the host sums the two head-group partial projections,
adds bproj, and transposes back.

v2 structure (vs v1): software-pipelined attention loop.  S matmuls are
row-tiled 4-way concurrent at full 512-col granularity; exp is split
ACT/DVE by a tunable per-j pattern; O+den accumulate into a fused 2-bank
PSUM tile via col-tiled matmuls with start=True first-write (no memsets);
O lags S by one j so PE never waits on exp; the per-qt tail (recip, onorm,
f32r projection, evac, DMA) is deferred into the next qt's first j slots.
Pre-phase: chunked input DMA, z-stream LN/KV first, then template attention
overlaps the x-stream conv/LN/KV; some LN elementwise runs on Pool.
"""
import os
import contextlib
import numpy as np

import concourse.bacc as bacc
import concourse.mybir as mybir
import concourse.tile as tile
from concourse.bass_utils import run_bass_kernel_spmd

F32 = mybir.dt.float32
F32R = mybir.dt.float32r
F16 = mybir.dt.float16
AF = mybir.ActivationFunctionType
OP = mybir.AluOpType

B, N, C = 4, 5120, 256
NHEADS, D, SR = 8, 32, 2
HZ = WZ = 32
HX = WX = 64
NZ, NX = HZ * WZ, HX * WX  # 1024, 4096
LZ, LX = (HZ // SR) * (WZ // SR), (HX // SR) * (WX // SR)  # 256, 1024
L = LZ + LX  # 1280
SCALE = float(D) ** -0.5
EPS = 1e-5
NCORES = 8
QTILE = 512
NQT = N // QTILE            # 10 query tiles (0,1 are template queries)
NJT = L // 128              # 10 key tiles (0,1 are template keys)
RECIP_C = 1.0 / C

_CACHED = {}

# degree-4 polynomial exp(SCALE*s) on s in [-4.59, 4.59] (scaled logits in
# [-0.81, 0.81]); p(0)=1 constrained LSQ on relative error, max rel err
# 4.3e-4 at range edge.  Coefficients folded with SCALE.
_EC = (0.99932575, 0.50072616, 0.17232145, 0.04077664)
EXP_C1 = _EC[0] * SCALE
EXP_C2 = _EC[1] * SCALE ** 2
EXP_C3 = _EC[2] * SCALE ** 3
EXP_C4 = _EC[3] * SCALE ** 4


def _register_exp_op():
    import concourse.dve_ops as dvo
    from concourse.dve_spec import (
        Spec, Src0, One, C0, C1, C2, C3, _spill_c3_to_src1, _has_src1, lower)
    from concourse.dve_uop import DveOpSpec
    name = "ANT_EXP_POLY4"
    for op in dvo.OPS:
        if op.name == name:
            return op
    body = _spill_c3_to_src1(
        One + Src0 * (C0 + Src0 * (C1 + Src0 * (C2 + Src0 * C3))))

    def _ref(in0, in1, s0, s1, imm2):
        c3 = np.asarray(in1).reshape(in1.shape[0], -1)[:, :1]
        return 1.0 + in0 * (s0 + in0 * (s1 + in0 * (imm2 + in0 * c3)))

    spec = Spec(body=body, reference=_ref)
    dvo._SUB_OPCODE_FOR_NAME[name] = dvo._CUSTOM_DVE_ROW_BASE + len(dvo.OPS)
    shas = {}
    for ver in ("v3", "v4"):
        s = DveOpSpec(name=name, opcode=dvo.get_dve_sub_opcode(name),
                      uops=lower(spec, ver=ver), rd1_en=_has_src1(spec))
        shas[ver] = s.sha(ver)
    op = dvo.DveOp(name, spec, subdim=False, uops_sha=shas)
    dvo.OPS.append(op)
    dvo.CUSTOM_DVE_SPECS[name] = spec
    return op


EXP_OP = _register_exp_op()


def _patch_act_tables():
    """Make every ACT function resolve to natural_log_exp_and_others.

    The table-load chooser assigns each activation the first set containing
    its function, so a kernel using Exp (attention) and Ln (rstd) bounces
    between two table sets -- one ACT_TABLE_LOAD + drain (~2.7us) per
    switch, several per iteration.  Emptying the earlier sets (positions
    preserved, so act_func_set_id indices stay valid) funnels Exp, Ln,
    Identity, Square and Copy into the single set that has them all; the
    fixpoint pass then hoists the one load out of the loop.
    """
    from concourse import hw_specs
    orig = hw_specs.get_activation_tables
    if getattr(bacc.get_activation_tables, "_ant_patched", False):
        return

    def gat(arch):
        t = orig(arch)
        out = {}
        for k, v in t.items():
            if k == "natural_log_exp_and_others":
                out[k] = v
            elif k in ("exp_and_others", "softplus_and_others",
                       "sigmoid_and_others", "sqrt_and_others", "small",
                       "natural_log"):
                out[k] = type(v)()
            else:
                out[k] = v
        return out
    gat._ant_patched = True
    bacc.get_activation_tables = gat


_patch_act_tables()

# per-j exp engine codes, cycled: digit = number of heads (of 4) whose
# exp runs on ACT this j; the rest run on the DVE poly op.  "2" = 50/50.
EXPP = os.environ.get("EXPP", "2")
EXPB = os.environ.get("EXPB", "3")  # override code for first j of each qt
POOL_LN = os.environ.get("POOL_LN", "1") == "1"   # LN elementwise on Pool
CHUNK_DMA = os.environ.get("CHUNK_DMA", "1") == "1"
TMPL_EARLY = os.environ.get("TMPL_EARLY", "1") == "1"
S_BUFS = int(os.environ.get("S_BUFS", "2"))
RECIP_ACT = os.environ.get("RECIP_ACT", "0") == "1"  # blocked: ACT Reciprocal
RSQRT_ACT = os.environ.get("RSQRT_ACT", "0") == "1"  # blocked: ACT Rsqrt
RSTD_FAST = os.environ.get("RSTD_FAST", "1") == "1"  # 1-pass DVE recip for rstd
LNEXP_RSTD = os.environ.get("LNEXP_RSTD", "1") == "1"  # rstd via ACT ln+exp
TAIL_CARRY = os.environ.get("TAIL_CARRY", "1") == "1"
YSB = os.environ.get("YSB", "A")  # proj evac: A=ACT, D=DVE, S=split
QEV = os.environ.get("QEV", "D")  # qt16 evac engine
KVE = os.environ.get("KVE", "D")  # kt16/v16 evac engine
CVE = os.environ.get("CVE", "D")  # conv y16 evac engine


def _build_nc(repeat=1):
    nc = bacc.Bacc("TRN2", target_bir_lowering=False)

    xT_d = nc.declare_dram_parameter("xT", [C, N], F32R, isOutput=False)
    wq_d = nc.declare_dram_parameter("wq", [C, 128], F32R, isOutput=False)
    wsr_d = nc.declare_dram_parameter("wsr", [8, 128, C], F32R, isOutput=False)
    wk_d = nc.declare_dram_parameter("wk", [C, 128], F16, isOutput=False)
    wv_d = nc.declare_dram_parameter("wv", [C, 128], F16, isOutput=False)
    wp_d = nc.declare_dram_parameter("wp", [128, C], F16, isOutput=False)
    lnp_d = nc.declare_dram_parameter("lnp", [C, 3], F32, isOutput=False)
    yT_d = nc.declare_dram_parameter("yT", [C, N], F32, isOutput=True)

    with tile.TileContext(nc) as tc, contextlib.ExitStack() as ctx:
        const = ctx.enter_context(tc.tile_pool(name="const", bufs=1))
        big = ctx.enter_context(tc.tile_pool(name="big", bufs=1))

        # ---- load weights ----
        wq_t = const.tile([128, 2, 128], F32R)
        nc.sync.dma_start(out=wq_t, in_=wq_d[:, :].rearrange("(c p) m -> p c m", p=128))
        wsr_t = const.tile([128, 8, C], F32R)
        nc.sync.dma_start(out=wsr_t, in_=wsr_d[:, :, :].rearrange("k p m -> p k m"))
        wk_t = const.tile([128, 2, 128], F16)
        nc.sync.dma_start(out=wk_t, in_=wk_d[:, :].rearrange("(c p) m -> p c m", p=128))
        wv_t = const.tile([128, 2, 128], F16)
        nc.sync.dma_start(out=wv_t, in_=wv_d[:, :].rearrange("(c p) m -> p c m", p=128))
        wp_t = const.tile([128, C], F16)
        nc.sync.dma_start(out=wp_t, in_=wp_d[:, :])
        lnp_t = const.tile([128, 2, 3], F32)
        nc.sync.dma_start(out=lnp_t, in_=lnp_d[:, :].rearrange("(c p) k -> p c k", p=128))
        ones16 = const.tile([128, 128], F16)
        nc.vector.memset(ones16, 1.0)
        onesC = const.tile([128, 128], F16)
        nc.vector.memset(onesC, RECIP_C)
        eps_t = const.tile([128, 1], F32)
        nc.vector.memset(eps_t, EPS)
        ec4_t = const.tile([128, 1], F32)
        nc.vector.memset(ec4_t, EXP_C4)
        # warm the ACT table set on the loop-entry path so the in-loop
        # fixpoint sees it loaded on every predecessor and hoists the load.
        actwarm = const.tile([128, 1], F32)
        nc.scalar.activation(actwarm, ec4_t, AF.Exp, scale=0.0)

        xT = big.tile([128, 2, N], F32R)
        qt16 = big.tile([128, N], F16)
        y16 = big.tile([128, 2, L], F16)
        catn16 = big.tile([128, 2, L], F16)
        kt16 = big.tile([128, L], F16)
        v16 = big.tile([128, NJT, 128], F16)

        # PSUM: s 2x[128,2,512] (4 banks) + od (2) + pp (2) = 8 banks
        ps = ctx.enter_context(tc.tile_pool(name="ps", bufs=1, space="PSUM"))
        sb = ctx.enter_context(tc.tile_pool(name="sbw", bufs=1))
        p_pool = ctx.enter_context(tc.tile_pool(name="p16", bufs=3))
        w_pool = ctx.enter_context(tc.tile_pool(name="work", bufs=2))

        env = dict(locals())
        if repeat == 1:
            _run_body(nc, tc, ctx, env)
        else:
            hints = (mybir.EngineType.PE, mybir.EngineType.Activation,
                     mybir.EngineType.DVE, mybir.EngineType.SP,
                     mybir.EngineType.Pool)
            with tc.For_i(0, repeat, 1, hint_engines=hints):
                _run_body(nc, tc, ctx, env, loop_mode=True)
            if TAIL_CARRY:
                od_f = env["_ps_tile"]("od", 1)
                env["_tail_now"](NQT - 1, od_f)
                nc.vector.memset(od_f[:, :, 0:1], 0.0)
    nc.compile()
    return nc


def _run_body(nc, tc, ctx, env, loop_mode=False):
    xT = env["xT"]; qt16 = env["qt16"]; y16 = env["y16"]; catn16 = env["catn16"]
    kt16 = env["kt16"]; v16 = env["v16"]; wq_t = env["wq_t"]; wsr_t = env["wsr_t"]
    wk_t = env["wk_t"]; wv_t = env["wv_t"]; wp_t = env["wp_t"]; lnp_t = env["lnp_t"]
    ones16 = env["ones16"]; onesC = env["onesC"]; eps_t = env["eps_t"]
    ec4_t = env["ec4_t"]; xT_d = env["xT_d"]; yT_d = env["yT_d"]
    ps = env["ps"]; sb = env["sb"]; p_pool = env["p_pool"]; w_pool = env["w_pool"]

    def ps_tile(tag, bufs):
        return ps.tile([128, 2, QTILE], F32, tag=tag, bufs=bufs, name=tag)

    # ---- input DMA (chunked so Q/conv start early; next iteration's
    # chunks overlap this iteration's attention) ----
    if CHUNK_DMA:
        for n0 in range(0, N, 1024):
            for cc in range(2):
                nc.sync.dma_start(out=xT[:, cc, n0:n0 + 1024],
                                  in_=xT_d[cc * 128:(cc + 1) * 128, n0:n0 + 1024])
    else:
        nc.sync.dma_start(out=xT[:, 0, :], in_=xT_d[0:128, :])
        nc.sync.dma_start(out=xT[:, 1, :], in_=xT_d[128:256, :])

    # ---- pre-phase helpers ----
    def q_proj(nt):
        qp = ps_tile("s", S_BUFS)
        for cc in range(2):
            nc.tensor.matmul(qp[:, 0, :], wq_t[:, cc, :],
                             xT[:, cc, nt * QTILE:(nt + 1) * QTILE],
                             start=(cc == 0), stop=(cc == 1))
        if QEV == "D":
            nc.vector.tensor_copy(qt16[:, nt * QTILE:(nt + 1) * QTILE], qp[:, 0, :])
        else:
            nc.scalar.copy(qt16[:, nt * QTILE:(nt + 1) * QTILE], qp[:, 0, :])

    imgz = xT[:, :, :NZ].rearrange("p c (i j) -> p c i j", i=HZ)
    imgx = xT[:, :, NZ:].rearrange("p c (i j) -> p c i j", i=HX)

    def conv_part(mt, part):
        cps = ps_tile("s", S_BUFS)
        if part == 'z':
            zps = cps[:, 0, :LZ]
            for k8 in range(8):
                kh, kw, cc = k8 >> 2, (k8 >> 1) & 1, k8 & 1
                rhs = imgz[:, cc, kh::2, kw::2]
                nc.tensor.matmul(zps, wsr_t[:, k8, mt * 128:(mt + 1) * 128], rhs,
                                 start=(k8 == 0), stop=(k8 == 7))
            if CVE == "D":
                nc.vector.tensor_scalar(y16[:, mt, 0:LZ], zps,
                                        lnp_t[:, mt, 0:1], None, OP.add)
            else:
                nc.scalar.activation(y16[:, mt, 0:LZ], zps, AF.Identity,
                                     bias=lnp_t[:, mt, 0:1])
        else:
            xt = part
            xps = cps[:, 0, :]
            for k8 in range(8):
                kh, kw, cc = k8 >> 2, (k8 >> 1) & 1, k8 & 1
                rhs = imgx[:, cc, 32 * xt + kh: 32 * xt + kh + 31: 2, kw::2]
                nc.tensor.matmul(xps, wsr_t[:, k8, mt * 128:(mt + 1) * 128], rhs,
                                 start=(k8 == 0), stop=(k8 == 7))
            if CVE == "D":
                nc.vector.tensor_scalar(
                    y16[:, mt, LZ + QTILE * xt: LZ + QTILE * (xt + 1)], xps,
                    lnp_t[:, mt, 0:1], None, OP.add)
            else:
                nc.scalar.activation(
                    y16[:, mt, LZ + QTILE * xt: LZ + QTILE * (xt + 1)],
                    xps, AF.Identity, bias=lnp_t[:, mt, 0:1])

    # LN scratch (SBUF, full L; segments written independently)
    ysq16 = sb.tile([128, 2, L], F16, tag="ysq")
    mean_b = sb.tile([128, L], F32, tag="mean")
    var_b = sb.tile([128, L], F32, tag="var")
    msq_b = sb.tile([128, L], F32, tag="msq")
    std_b = sb.tile([128, L], F32, tag="std")
    rstd_b = sb.tile([128, L], F32, tag="rstd")
    rscr_b = sb.tile([128, L], F32, tag="rscr")

    def ln_segment(off, sz):
        sl = slice(off, off + sz)
        for cc in range(2):
            if POOL_LN:
                nc.gpsimd.tensor_mul(ysq16[:, cc, sl], y16[:, cc, sl], y16[:, cc, sl])
            else:
                nc.scalar.square(ysq16[:, cc, sl], y16[:, cc, sl])
        st = ps_tile("s", S_BUFS)
        for cc in range(2):
            nc.tensor.matmul(st[:, 0, :sz], onesC, y16[:, cc, sl],
                             start=(cc == 0), stop=(cc == 1))
            nc.tensor.matmul(st[:, 1, :sz], onesC, ysq16[:, cc, sl],
                             start=(cc == 0), stop=(cc == 1))
        nc.scalar.copy(mean_b[:, sl], st[:, 0, :sz])
        nc.scalar.copy(var_b[:, sl], st[:, 1, :sz])
        if POOL_LN:
            nc.gpsimd.tensor_mul(msq_b[:, sl], mean_b[:, sl], mean_b[:, sl])
            nc.gpsimd.tensor_tensor(var_b[:, sl], var_b[:, sl], msq_b[:, sl],
                                    OP.subtract)
        else:
            nc.vector.tensor_mul(msq_b[:, sl], mean_b[:, sl], mean_b[:, sl])
            nc.vector.tensor_tensor(var_b[:, sl], var_b[:, sl], msq_b[:, sl],
                                    OP.subtract)
        if LNEXP_RSTD:
            # rstd = exp(-0.5*ln(var+eps)): keeps ACT on the
            # natural_log_exp_and_others table set (no per-iteration
            # ACT_TABLE_LOAD switches between Sqrt and the attention Exp).
            nc.scalar.activation(std_b[:, sl], var_b[:, sl], AF.Ln,
                                 bias=eps_t[:, 0:1])
            nc.scalar.activation(rstd_b[:, sl], std_b[:, sl], AF.Exp,
                                 scale=-0.5)
        else:
            nc.scalar.activation(std_b[:, sl], var_b[:, sl], AF.Sqrt,
                                 bias=eps_t[:, 0:1])
            if RSTD_FAST:
                nc.vector.reciprocal_approx_fast(rstd_b[:, sl], std_b[:, sl])
            else:
                nc.vector.reciprocal_approx_accurate(
                    rstd_b[:, sl], std_b[:, sl], rscr_b[:, sl])
        for cc in range(2):
            t32 = sb.tile([128, QTILE], F32, tag="t32")
            if POOL_LN:
                nc.gpsimd.tensor_tensor(t32[:, :sz], y16[:, cc, sl],
                                        mean_b[:, sl], OP.subtract)
                nc.gpsimd.tensor_tensor(t32[:, :sz], t32[:, :sz],
                                        rstd_b[:, sl], OP.mult)
            else:
                nc.vector.tensor_tensor(t32[:, :sz], y16[:, cc, sl],
                                        mean_b[:, sl], OP.subtract)
                nc.vector.tensor_tensor(t32[:, :sz], t32[:, :sz],
                                        rstd_b[:, sl], OP.mult)
            nc.scalar.activation(catn16[:, cc, sl], t32[:, :sz], AF.Identity,
                                 bias=lnp_t[:, cc, 2:3],
                                 scale=lnp_t[:, cc, 1:2])

    def k_segment(off, sz):
        sl = slice(off, off + sz)
        kps = ps_tile("s", S_BUFS)
        for cc in range(2):
            nc.tensor.matmul(kps[:, 0, :sz], wk_t[:, cc, :], catn16[:, cc, sl],
                             start=(cc == 0), stop=(cc == 1))
        if KVE == "D":
            nc.vector.tensor_copy(kt16[:, sl], kps[:, 0, :sz])
        else:
            nc.scalar.copy(kt16[:, sl], kps[:, 0, :sz])

    def v_tile(jt):
        vps = ps_tile("s", S_BUFS)
        for cc in range(2):
            nc.tensor.matmul(vps[:, 0, :128],
                             catn16[:, cc, jt * 128:(jt + 1) * 128],
                             wv_t[:, cc, :],
                             start=(cc == 0), stop=(cc == 1))
        if KVE == "D":
            nc.vector.tensor_copy(v16[:, jt, :], vps[:, 0, :128])
        else:
            nc.scalar.copy(v16[:, jt, :], vps[:, 0, :128])

    # ---- attention machinery (software-pipelined) ----
    # tail(qt) is emitted in two chunks during the next qt's first j slots.
    pending_tail = []

    def emit_tail_chunk():
        if pending_tail:
            pending_tail.pop(0)()

    def t_recip_for(qt, od):
        recip = w_pool.tile([128, QTILE], F32, tag="recip", name="recip")
        onorm = w_pool.tile([128, QTILE], F16, tag="onorm", name="onorm")
        nc.vector.reciprocal_approx_fast(recip, od[:, 1, :])
        nc.vector.tensor_mul(onorm, od[:, 0, :], recip)
        return onorm

    def t_proj_for(qt, onorm):
        pp = ps_tile("pp", 1)
        for mt in range(2):
            nc.tensor.matmul(pp[:, mt, :], wp_t[:, mt * 128:(mt + 1) * 128],
                             onorm, start=True, stop=True)
        ysb = w_pool.tile([128, 2, QTILE], F32, tag="ysb", name="ysb")
        if YSB == "A":
            nc.scalar.copy(ysb, pp)
        elif YSB == "D":
            nc.vector.tensor_copy(ysb, pp)
        else:
            nc.scalar.copy(ysb[:, 0, :], pp[:, 0, :])
            nc.vector.tensor_copy(ysb[:, 1, :], pp[:, 1, :])
        nc.sync.dma_start(
            out=yT_d[:, qt * QTILE:(qt + 1) * QTILE].rearrange(
                "(c p) m -> p c m", p=128),
            in_=ysb)

    def tail_now(qt, od):
        t_proj_for(qt, t_recip_for(qt, od))

    def make_tail(qt, od):
        if loop_mode and TAIL_CARRY and qt == NQT - 1:
            # loop-carried: the last qt's tail runs at the TOP of the next
            # iteration (same od buffer, bufs=1), overlapping the input DMA
            # and pre-phase; _build_nc emits a final drain after the loop.
            return
        state = {}

        def t_recip():
            state["onorm"] = t_recip_for(qt, od)

        def t_proj():
            t_proj_for(qt, state["onorm"])
        pending_tail.append(t_recip)
        pending_tail.append(t_proj)
    env["_tail_now"] = tail_now
    env["_ps_tile"] = ps_tile

    if loop_mode and TAIL_CARRY:
        od_c = ps_tile("od", 1)
        tail_now(NQT - 1, od_c)
        # the allocator requires every tile generation to have a write; the
        # slot is fully overwritten by qt0's start=True accumulation anyway.
        nc.vector.memset(od_c[:, :, 0:1], 0.0)

    def attn_qt(qt):
        jts = list(range(2)) if qt < 2 else list(range(NJT))
        qs = slice(qt * QTILE, (qt + 1) * QTILE)
        od = ps_tile("od", 1)
        prev = None

        def emit_O(jt, pA, pB, first, last):
            for h in range(4):
                p_h = pA[:, h, :] if h < 2 else pB[:, h - 2, :]
                nc.tensor.matmul(od[32 * h:32 * h + 32, 0, :],
                                 v16[:, jt, 32 * h:32 * h + 32], p_h,
                                 start=first, stop=last,
                                 tile_position=(0, 32 * h),
                                 skip_group_check=True)
            for h in range(4):
                p_h = pA[:, h, :] if h < 2 else pB[:, h - 2, :]
                nc.tensor.matmul(od[32 * h:32 * h + 32, 1, :],
                                 ones16[:, 0:32], p_h,
                                 start=first, stop=last,
                                 tile_position=(0, 32 * h),
                                 skip_group_check=True)

        for idx, jt in enumerate(jts):
            js = slice(jt * 128, (jt + 1) * 128)
            sA = ps_tile("s", S_BUFS)
            sB = ps_tile("s", S_BUFS)
            for h in (0, 1):
                nc.tensor.matmul(sA[:, h, :], kt16[32 * h:32 * h + 32, js],
                                 qt16[32 * h:32 * h + 32, qs],
                                 start=True, stop=True,
                                 tile_position=(32 * h, 0))
            for h in (2, 3):
                nc.tensor.matmul(sB[:, h - 2, :], kt16[32 * h:32 * h + 32, js],
                                 qt16[32 * h:32 * h + 32, qs],
                                 start=True, stop=True,
                                 tile_position=(32 * h, 0))
            a = int(EXPP[attn_qt.jcount % len(EXPP)])
            if idx == 0 and EXPB:
                a = int(EXPB)
            attn_qt.jcount += 1
            pA = p_pool.tile([128, 2, QTILE], F16, tag="pA", name="pA")
            pB = p_pool.tile([128, 2, QTILE], F16, tag="pB", name="pB")
            for p2, s2, na in ((pA, sA, min(a, 2)), (pB, sB, max(a - 2, 0))):
                if na > 0:
                    nc.scalar.activation(p2[:, :na, :], s2[:, :na, :],
                                         AF.Exp, scale=SCALE)
                if na < 2:
                    nc.vector._custom_dve(
                        EXP_OP, out=p2[:, na:, :], in0=s2[:, na:, :],
                        in1=ec4_t, s0=EXP_C1, s1=EXP_C2, imm2=EXP_C3)
            if idx == 0:
                emit_tail_chunk()
            if prev is not None:
                emit_O(*prev, first=(idx == 1), last=False)
                if idx == 1:
                    emit_tail_chunk()
            prev = (jt, pA, pB)
        emit_O(*prev, first=(len(jts) == 1), last=True)
        make_tail(qt, od)

    attn_qt.jcount = 0

    # ---- emission schedule ----
    # z-stream pre first, then template attention overlapping x-stream pre.
    q_proj(0)
    q_proj(1)
    for mt in range(2):
        conv_part(mt, 'z')
    ln_segment(0, LZ)
    k_segment(0, LZ)
    v_tile(0)
    v_tile(1)
    for nt in range(2, NQT):
        q_proj(nt)
    if TMPL_EARLY:
        attn_qt(0)
        attn_qt(1)
    for part in (0, 1):
        for mt in range(2):
            conv_part(mt, part)
    ln_segment(LZ, QTILE)
    ln_segment(LZ + QTILE, QTILE)
    k_segment(LZ, QTILE)
    k_segment(LZ + QTILE, QTILE)
    for jt in range(2, NJT):
        v_tile(jt)
    if not TMPL_EARLY:
        attn_qt(0)
        attn_qt(1)
    for qt in range(2, NQT):
        attn_qt(qt)
    while pending_tail:
        emit_tail_chunk()


def _get_nc():
    if "nc" not in _CACHED:
        _CACHED["nc"] = _build_nc()
    return _CACHED["nc"]


def _prep_inputs(x, Wq, Wkv, Wsr, bsr, gamma, beta, Wproj, bproj):
    """Build the 8 per-core input dicts (host-side shard + transpose)."""
    x = np.asarray(x, np.float32)
    Wq = np.asarray(Wq, np.float32)
    Wkv = np.asarray(Wkv, np.float32)
    Wsr = np.asarray(Wsr, np.float32)
    Wproj = np.asarray(Wproj, np.float32)
    lnp = np.ascontiguousarray(
        np.stack([np.asarray(bsr, np.float32), np.asarray(gamma, np.float32),
                  np.asarray(beta, np.float32)], axis=1))
    wsr8 = np.ascontiguousarray(
        Wsr.transpose(2, 3, 1, 0).reshape(8, 128, C).astype(np.float32))
    in_maps = []
    for core in range(NCORES):
        b, hg = core // 2, core % 2
        sl = slice(hg * 128, (hg + 1) * 128)
        in_maps.append({
            "xT": np.ascontiguousarray(x[b].T),
            "wq": np.ascontiguousarray(Wq[sl, :].T),
            "wsr": wsr8,
            "wk": np.ascontiguousarray(Wkv[:C][sl, :].T.astype(np.float16)),
            "wv": np.ascontiguousarray(Wkv[C:][sl, :].T.astype(np.float16)),
            "wp": np.ascontiguousarray(Wproj[:, sl].T.astype(np.float16)),
            "lnp": lnp,
        })
    return in_maps


def kernel(x, Wq, Wkv, Wsr, bsr, gamma, beta, Wproj, bproj,
           H_x=64, W_x=64, H_z=32, W_z=32, _trace=False, _trace_kwargs=None):
    assert int(H_x) == HX and int(W_x) == WX and int(H_z) == HZ and int(W_z) == WZ
    nc = _get_nc()
    in_maps = _prep_inputs(x, Wq, Wkv, Wsr, bsr, gamma, beta, Wproj, bproj)
    kw = dict(_trace_kwargs or {})
    res = run_bass_kernel_spmd(nc, in_maps, core_ids=list(range(NCORES)),
                               trace=_trace, **kw)
    _CACHED["last_result"] = res
    bproj = np.asarray(bproj, np.float32)
    out = np.empty((B, N, C), np.float32)
    for b in range(B):
        yT = res.results[2 * b]["yT"] + res.results[2 * b + 1]["yT"]
        out[b] = yT.T + bproj
    return out

